# revision 1
# baseline (speedup 1.0000x reference)
"""BertSelfAttention on 8 Trainium2 NeuronCores.

Sharding: data parallel over batch (B=2) x tensor parallel over heads
(16 heads -> 4 groups of 4). Core c handles batch c//4, heads 4*(c%4)..+4.
No collectives needed: each core produces a disjoint [2048, 256] slice of
the output which the host concatenates.

Per-core device program (identical on all cores, SPMD over data):
  inputs (host-prepped):
    xt    [1024, 2048]  hidden_states[b].T
    wq/wk/wv [1024, 256] weight column slices (wq,qb pre-scaled by 1/8)
    qb2/kb2 [128, 2]    bias chunks (per-partition layout)
    vb    [1, 256]
    maskc [128, 16]     additive mask chunks (mask[c*128+p] at [p, c])
  output:
    out   [2048, 256]   context slice, token-major

  Stage A (projections, all on PE with K=128 contractions):
    Q.T, K.T feature-major  [128 feats(2 heads), 2048 tokens]
    V token-major [128 tokens x 16 tiles, 4*65] with a ones column per head
  Stage B (attention per head, q processed in 512-wide blocks):
    S_T[k, q] = K_h.T-tile @ Q_h   (PE, contraction over head_dim=64)
    expS = exp(S_T + mask_k)       (ACT, mask as per-partition bias)
    ctxT/denom = V_aug.T-tile @ expS accumulated over k  (PE, M=65:
                 rows 0-63 unnormalized ctx.T, row 64 = softmax denom)
    PE-transpose [65,128] tiles -> [q, 65]; DVE reciprocal + per-partition
    multiply completes the softmax normalization; result lands token-major.
"""

import numpy as np

HIDDEN = 1024
HEADS = 16
HD = 64
B = 2
S = 2048
NCORES = 8
HPC = HEADS // 4  # heads per core = 4
WCOLS = HPC * HD  # 256 weight columns per core

_CACHE = {}


def _build_program():
    import concourse.bacc as bacc
    import concourse.tile as tile
    import concourse.mybir as mybir
    from concourse.masks import make_identity

    f32 = mybir.dt.float32

    nc = bacc.Bacc("TRN2", target_bir_lowering=False, debug=False, num_devices=NCORES)

    xt_d = nc.dram_tensor("xt", [HIDDEN, S], f32, kind="ExternalInput")
    wq_d = nc.dram_tensor("wq", [HIDDEN, WCOLS], f32, kind="ExternalInput")
    wk_d = nc.dram_tensor("wk", [HIDDEN, WCOLS], f32, kind="ExternalInput")
    wv_d = nc.dram_tensor("wv", [HIDDEN, WCOLS], f32, kind="ExternalInput")
    qb_d = nc.dram_tensor("qb2", [128, 2], f32, kind="ExternalInput")
    kb_d = nc.dram_tensor("kb2", [128, 2], f32, kind="ExternalInput")
    vb_d = nc.dram_tensor("vb", [1, WCOLS], f32, kind="ExternalInput")
    mask_d = nc.dram_tensor("maskc", [128, 16], f32, kind="ExternalInput")
    out_d = nc.dram_tensor("out", [S, WCOLS], f32, kind="ExternalOutput")

    xt_r = xt_d.ap().rearrange("(c p) m -> p c m", p=128)  # [128, 8, 2048]
    wq_r = wq_d.ap().rearrange("(c p) n -> p c n", p=128)  # [128, 8, 256]
    wk_r = wk_d.ap().rearrange("(c p) n -> p c n", p=128)
    wv_r = wv_d.ap().rearrange("(c p) n -> p c n", p=128)
    out_r = out_d.ap().rearrange("(t p) n -> p t n", p=128)  # [128, 16, 256]

    with tile.TileContext(nc) as tc:
        with tc.tile_pool(name="persist", bufs=1) as persist:
            # persistent SBUF
            q_sb = persist.tile([128, 2, S], f32)  # [feat(2 heads), mc, token]
            k_sb = persist.tile([128, 2, S], f32)
            v_sb = persist.tile([128, 16, 4 * 65], f32)  # [token, tile, 4*(64+ones)]
            o_sb = persist.tile([128, 16, WCOLS], f32)  # [token, tile, feat]
            qb_sb = persist.tile([128, 2], f32)
            kb_sb = persist.tile([128, 2], f32)
            vb_sb = persist.tile([1, WCOLS], f32)
            mask_sb = persist.tile([128, 16], f32)
            ones_sb = persist.tile([1, 128], f32)
            ident = persist.tile([65, 65], f32)

            nc.sync.dma_start(out=qb_sb[:], in_=qb_d.ap())
            nc.sync.dma_start(out=kb_sb[:], in_=kb_d.ap())
            nc.sync.dma_start(out=vb_sb[:], in_=vb_d.ap())
            nc.sync.dma_start(out=mask_sb[:], in_=mask_d.ap())
            nc.vector.memset(ones_sb[:], 1.0)
            make_identity(nc, ident[:])
            # ones columns of V (one per head): cols 64, 129, 194, 259
            v_blk = v_sb.rearrange("p m (l c) -> p m l c", l=4)
            nc.vector.memset(v_blk[:, :, :, 64:65], 1.0)

            # ---------------- Stage A: projections ----------------
            with (
                tc.tile_pool(name="proj", bufs=1) as proj,
                tc.tile_pool(name="psA", bufs=3, space="PSUM") as psA,
                tc.tile_pool(name="psV", bufs=2, space="PSUM") as psV,
            ):
                xt = proj.tile([128, 8, S], f32)
                wq_sb = proj.tile([128, 8, WCOLS], f32)
                wk_sb = proj.tile([128, 8, WCOLS], f32)
                wv_sb = proj.tile([128, 8, WCOLS], f32)
                for k in range(8):
                    nc.sync.dma_start(out=xt[:, k, :], in_=xt_r[:, k, :])
                nc.sync.dma_start(out=wq_sb[:], in_=wq_r)
                nc.sync.dma_start(out=wk_sb[:], in_=wk_r)
                nc.sync.dma_start(out=wv_sb[:], in_=wv_r)

                # Q.T / K.T: [feat, token], feature-chunk mc, token-span sp
                for w_sb, b_sb, dst in ((wq_sb, qb_sb, q_sb), (wk_sb, kb_sb, k_sb)):
                    for mc in range(2):
                        for sp in range(4):
                            pq = psA.tile([128, 512], f32, tag="pq")
                            for k in range(8):
                                nc.tensor.matmul(
                                    pq[:],
                                    lhsT=w_sb[:, k, mc * 128 : mc * 128 + 128],
                                    rhs=xt[:, k, sp * 512 : sp * 512 + 512],
                                    start=(k == 0),
                                    stop=(k == 7),
                                )
                            nc.vector.tensor_scalar_add(
                                dst[:, mc, sp * 512 : sp * 512 + 512],
                                pq[:],
                                b_sb[:, mc : mc + 1],
                            )

                # V: token-major [token, feat]
                for mt in range(16):
                    pv = psV.tile([128, WCOLS], f32, tag="pv")
                    for k in range(8):
                        nc.tensor.matmul(
                            pv[:],
                            lhsT=xt[:, k, mt * 128 : mt * 128 + 128],
                            rhs=wv_sb[:, k, :],
                            start=(k == 0),
                            stop=False,
                        )
                    nc.tensor.matmul(
                        pv[:],
                        lhsT=ones_sb[0:1, 0:128],
                        rhs=vb_sb[0:1, :],
                        start=False,
                        stop=True,
                    )
                    for lh in range(4):
                        nc.vector.tensor_copy(
                            v_sb[:, mt, 65 * lh : 65 * lh + 64],
                            pv[:, 64 * lh : 64 * lh + 64],
                        )

            # ---------------- Stage B: attention ----------------
            with (
                tc.tile_pool(name="att", bufs=2) as att,
                tc.tile_pool(name="wk", bufs=4) as wkp,
                tc.tile_pool(name="ps_s", bufs=3, space="PSUM") as ps_s,
                tc.tile_pool(name="ps_c", bufs=2, space="PSUM") as ps_c,
                tc.tile_pool(name="ps_t", bufs=2, space="PSUM") as ps_t,
            ):
                for lh in range(4):
                    mc = lh // 2
                    rs = 64 * (lh % 2)
                    for blk in range(4):  # 512-wide q block
                        qs = blk * 512
                        # expS[k, q-block]: 16 k-tiles x 512 q
                        expS = att.tile([128, 16, 512], f32, tag="expS")
                        for kt in range(16):
                            ps = ps_s.tile([128, 512], f32, tag="ps")
                            nc.tensor.matmul(
                                ps[:],
                                lhsT=k_sb[rs : rs + 64, mc, kt * 128 : kt * 128 + 128],
                                rhs=q_sb[rs : rs + 64, mc, qs : qs + 512],
                                start=True,
                                stop=True,
                            )
                            nc.scalar.activation(
                                expS[:, kt, :],
                                ps[:],
                                mybir.ActivationFunctionType.Exp,
                                bias=mask_sb[:, kt : kt + 1],
                            )
                        # ctx.T (+denominator in row 64), accumulated over k
                        pc = ps_c.tile([65, 512], f32, tag="pc")
                        for kt in range(16):
                            nc.tensor.matmul(
                                pc[:],
                                lhsT=v_sb[:, kt, 65 * lh : 65 * lh + 65],
                                rhs=expS[:, kt, :],
                                start=(kt == 0),
                                stop=(kt == 15),
                            )
                        ctxT = wkp.tile([65, 512], f32, tag="ctxT")
                        nc.vector.tensor_copy(ctxT[:], pc[:])
                        # transpose to token-major + normalize
                        for tt in range(4):
                            qt = blk * 4 + tt
                            pt = ps_t.tile([128, 65], f32, tag="pt")
                            nc.tensor.transpose(
                                pt[:],
                                ctxT[:, tt * 128 : tt * 128 + 128],
                                ident[:],
                            )
                            rcol = wkp.tile([128, 1], f32, tag="rcol")
                            nc.vector.reciprocal(rcol[:], pt[:, 64:65])
                            nc.vector.tensor_scalar_mul(
                                o_sb[:, qt, 64 * lh : 64 * lh + 64],
                                pt[:, 0:64],
                                rcol[:],
                            )

                for qt in range(16):
                    nc.sync.dma_start(out=out_r[:, qt, :], in_=o_sb[:, qt, :])

    nc.compile()
    return nc


def _get_program():
    if "nc" not in _CACHE:
        _CACHE["nc"] = _build_program()
    return _CACHE["nc"]


def kernel(hidden_states, attention_mask, q_w, q_b, k_w, k_b, v_w, v_b):
    from concourse import bass_utils

    nc = _get_program()

    hs = np.asarray(hidden_states, np.float32)
    am = np.asarray(attention_mask, np.float32)
    q_w = np.asarray(q_w, np.float32)
    k_w = np.asarray(k_w, np.float32)
    v_w = np.asarray(v_w, np.float32)
    q_b = np.asarray(q_b, np.float32)
    k_b = np.asarray(k_b, np.float32)
    v_b = np.asarray(v_b, np.float32)

    scale = np.float32(1.0 / np.sqrt(HD))

    in_maps = []
    for c in range(NCORES):
        b = c // 4
        hg = c % 4
        cols = slice(WCOLS * hg, WCOLS * hg + WCOLS)
        mask = am[b, 0, 0, :]  # [S]
        in_maps.append(
            {
                "xt": np.ascontiguousarray(hs[b].T),
                "wq": np.ascontiguousarray(q_w[:, cols] * scale),
                "wk": np.ascontiguousarray(k_w[:, cols]),
                "wv": np.ascontiguousarray(v_w[:, cols]),
                "qb2": np.ascontiguousarray((q_b[cols] * scale).reshape(2, 128).T),
                "kb2": np.ascontiguousarray(k_b[cols].reshape(2, 128).T),
                "vb": np.ascontiguousarray(v_b[cols].reshape(1, WCOLS)),
                "maskc": np.ascontiguousarray(mask.reshape(16, 128).T),
            }
        )

    res = bass_utils.run_bass_kernel_spmd(nc, in_maps, core_ids=list(range(NCORES)))

    full = np.empty((B, S, HIDDEN), np.float32)
    for c in range(NCORES):
        b = c // 4
        hg = c % 4
        full[b, :, WCOLS * hg : WCOLS * hg + WCOLS] = res.results[c]["out"]
    return full


# revision 5
# speedup vs baseline: 1.5238x; 1.5238x over previous
"""BertSelfAttention on 8 Trainium2 NeuronCores.

Sharding: data parallel over batch (B=2) x tensor parallel over heads
(16 heads -> 4 groups of 4). Core c handles batch c//4, heads 4*(c%4)..+4.
No collectives needed: each core produces a disjoint [2048, 256] slice of
the output which the host concatenates.

Per-core device program (identical on all cores, SPMD over data):
  inputs (host-prepped):
    xt    [1024, 2048]  hidden_states[b].T
    wq/wk/wv [1024, 256] weight column slices (wq,qb pre-scaled by 1/8)
    qb2/kb2 [128, 2]    bias chunks (per-partition layout)
    vb    [1, 256]
    maskc [128, 16]     additive mask chunks (mask[c*128+p] at [p, c])
  output:
    out   [2048, 256]   context slice, token-major

  Stage A (projections, all on PE with K=128 contractions):
    Q.T, K.T feature-major  [128 feats(2 heads), 2048 tokens]
    V token-major [128 tokens x 16 tiles, 4*65] with a ones column per head
  Stage B (attention per head, q processed in 512-wide blocks):
    S_T[k, q] = K_h.T-tile @ Q_h   (PE, contraction over head_dim=64)
    expS = exp(S_T + mask_k)       (ACT, mask as per-partition bias)
    ctxT/denom = V_aug.T-tile @ expS accumulated over k  (PE, M=65:
                 rows 0-63 unnormalized ctx.T, row 64 = softmax denom)
    PE-transpose [65,128] tiles -> [q, 65]; DVE reciprocal + per-partition
    multiply completes the softmax normalization; result lands token-major.
"""

import numpy as np

HIDDEN = 1024
HEADS = 16
HD = 64
B = 2
S = 2048
NCORES = 8
HPC = HEADS // 4  # heads per core = 4
WCOLS = HPC * HD  # 256 weight columns per core

_CACHE = {}


def _build_program():
    import concourse.bacc as bacc
    import concourse.tile as tile
    import concourse.mybir as mybir
    from concourse.masks import make_identity

    f32 = mybir.dt.float32
    f32r = mybir.dt.float32r
    r = lambda ap: ap.bitcast(f32r)

    nc = bacc.Bacc("TRN2", target_bir_lowering=False, debug=False, num_devices=NCORES)

    xt_d = nc.dram_tensor("xt", [HIDDEN, S], f32r, kind="ExternalInput")
    wq_d = nc.dram_tensor("wq", [HIDDEN, WCOLS], f32r, kind="ExternalInput")
    wk_d = nc.dram_tensor("wk", [HIDDEN, WCOLS], f32r, kind="ExternalInput")
    wv_d = nc.dram_tensor("wv", [HIDDEN, WCOLS], f32r, kind="ExternalInput")
    qb_d = nc.dram_tensor("qb2", [128, 2], f32, kind="ExternalInput")
    kb_d = nc.dram_tensor("kb2", [128, 2], f32, kind="ExternalInput")
    vb_d = nc.dram_tensor("vb", [1, WCOLS], f32, kind="ExternalInput")
    mask_d = nc.dram_tensor("maskc", [128, 16], f32, kind="ExternalInput")
    out_d = nc.dram_tensor("out", [S, WCOLS], f32, kind="ExternalOutput")

    xt_r = xt_d.ap().rearrange("(c p) m -> p c m", p=128)  # [128, 8, 2048]
    wq_r = wq_d.ap().rearrange("(c p) n -> p c n", p=128)  # [128, 8, 256]
    wk_r = wk_d.ap().rearrange("(c p) n -> p c n", p=128)
    wv_r = wv_d.ap().rearrange("(c p) n -> p c n", p=128)
    out_r = out_d.ap().rearrange("(t p) n -> p t n", p=128)  # [128, 16, 256]

    with tile.TileContext(nc) as tc:
        with tc.tile_pool(name="persist", bufs=1) as persist:
            # persistent SBUF
            q_sb = persist.tile([128, 2, S], f32r)  # [feat(2 heads), mc, token]
            k_sb = persist.tile([128, 2, S], f32r)
            v_sb = persist.tile([128, 16, 4 * 65], f32r)  # [token, tile, 4*(64+ones)]
            o_sb = persist.tile([128, 16, WCOLS], f32)  # [token, tile, feat]
            qb_sb = persist.tile([128, 2], f32)
            kb_sb = persist.tile([128, 2], f32)
            vb_sb = persist.tile([1, WCOLS], f32)
            mask_sb = persist.tile([128, 16], f32)
            ones_sb = persist.tile([1, 128], f32)
            ident = persist.tile([65, 65], f32)

            nc.sync.dma_start(out=qb_sb[:], in_=qb_d.ap())
            nc.sync.dma_start(out=kb_sb[:], in_=kb_d.ap())
            nc.sync.dma_start(out=vb_sb[:], in_=vb_d.ap())
            nc.sync.dma_start(out=mask_sb[:], in_=mask_d.ap())
            nc.vector.memset(ones_sb[:], 1.0)
            make_identity(nc, ident[:])
            # ones columns of V (one per head): cols 64, 129, 194, 259
            # (memset can't write f32r; round through a DVE copy instead)
            v_blk = v_sb.rearrange("p m (l c) -> p m l c", l=4)
            vst = persist.tile([128, 16, 4], f32)
            nc.vector.memset(vst[:], 1.0)
            nc.vector.tensor_copy(v_blk[:, :, :, 64], vst[:])

            # ---------------- Stage A: projections ----------------
            with (
                tc.tile_pool(name="proj", bufs=1) as proj,
                tc.tile_pool(name="psA", bufs=3, space="PSUM") as psA,
                tc.tile_pool(name="psV", bufs=2, space="PSUM") as psV,
            ):
                xt = proj.tile([128, 8, S], f32r)
                wq_sb = proj.tile([128, 8, WCOLS], f32r)
                wk_sb = proj.tile([128, 8, WCOLS], f32r)
                wv_sb = proj.tile([128, 8, WCOLS], f32r)
                for k in range(8):
                    nc.sync.dma_start(out=xt[:, k, :], in_=xt_r[:, k, :])
                nc.sync.dma_start(out=wq_sb[:], in_=wq_r)
                nc.sync.dma_start(out=wk_sb[:], in_=wk_r)
                nc.sync.dma_start(out=wv_sb[:], in_=wv_r)

                # Q.T / K.T: [feat, token], feature-chunk mc, token-span sp
                for w_sb, b_sb, dst in ((wq_sb, qb_sb, q_sb), (wk_sb, kb_sb, k_sb)):
                    for mc in range(2):
                        for sp in range(4):
                            pq = psA.tile([128, 512], f32, tag="pq")
                            for k in range(8):
                                nc.tensor.matmul(
                                    pq[:],
                                    lhsT=(w_sb[:, k, mc * 128 : mc * 128 + 128]),
                                    rhs=(xt[:, k, sp * 512 : sp * 512 + 512]),
                                    start=(k == 0),
                                    stop=(k == 7),
                                )
                            nc.vector.tensor_scalar_add(
                                dst[:, mc, sp * 512 : sp * 512 + 512],
                                pq[:],
                                b_sb[:, mc : mc + 1],
                            )

                # V: token-major [token, feat]
                for mt in range(16):
                    pv = psV.tile([128, WCOLS], f32, tag="pv")
                    for k in range(8):
                        nc.tensor.matmul(
                            pv[:],
                            lhsT=(xt[:, k, mt * 128 : mt * 128 + 128]),
                            rhs=(wv_sb[:, k, :]),
                            start=(k == 0),
                            stop=False,
                        )
                    nc.tensor.matmul(
                        pv[:],
                        lhsT=(ones_sb[0:1, 0:128]),
                        rhs=(vb_sb[0:1, :]),
                        start=False,
                        stop=True,
                    )
                    for lh in range(4):
                        nc.vector.tensor_copy(
                            v_sb[:, mt, 65 * lh : 65 * lh + 64],
                            pv[:, 64 * lh : 64 * lh + 64],
                        )

            # ---------------- Stage B: attention ----------------
            with (
                tc.tile_pool(name="att", bufs=2) as att,
                tc.tile_pool(name="wk", bufs=4) as wkp,
                tc.tile_pool(name="ps_s", bufs=3, space="PSUM") as ps_s,
                tc.tile_pool(name="ps_c", bufs=2, space="PSUM") as ps_c,
                tc.tile_pool(name="ps_t", bufs=2, space="PSUM") as ps_t,
            ):
                for lh in range(4):
                    mc = lh // 2
                    rs = 64 * (lh % 2)
                    for blk in range(4):  # 512-wide q block
                        qs = blk * 512
                        # expS[k, q-block]: 16 k-tiles x 512 q
                        expS = att.tile([128, 16, 512], f32r, tag="expS")
                        for kt in range(16):
                            ps = ps_s.tile([128, 512], f32, tag="ps")
                            nc.tensor.matmul(
                                ps[:],
                                lhsT=(k_sb[rs : rs + 64, mc, kt * 128 : kt * 128 + 128]),
                                rhs=(q_sb[rs : rs + 64, mc, qs : qs + 512]),
                                start=True,
                                stop=True,
                            )
                            nc.scalar.activation(
                                expS[:, kt, :],
                                ps[:],
                                mybir.ActivationFunctionType.Exp,
                                bias=mask_sb[:, kt : kt + 1],
                            )
                        # ctx.T (+denominator in row 64), accumulated over k
                        pc = ps_c.tile([65, 512], f32, tag="pc")
                        for kt in range(16):
                            nc.tensor.matmul(
                                pc[:],
                                lhsT=(v_sb[:, kt, 65 * lh : 65 * lh + 65]),
                                rhs=(expS[:, kt, :]),
                                start=(kt == 0),
                                stop=(kt == 15),
                            )
                        ctxT = wkp.tile([65, 512], f32, tag="ctxT")
                        nc.vector.tensor_copy(ctxT[:], pc[:])
                        # transpose to token-major + normalize
                        for tt in range(4):
                            qt = blk * 4 + tt
                            pt = ps_t.tile([128, 65], f32, tag="pt")
                            nc.tensor.transpose(
                                pt[:],
                                ctxT[:, tt * 128 : tt * 128 + 128],
                                ident[:],
                            )
                            rcol = wkp.tile([128, 1], f32, tag="rcol")
                            nc.vector.reciprocal(rcol[:], pt[:, 64:65])
                            nc.vector.tensor_scalar_mul(
                                o_sb[:, qt, 64 * lh : 64 * lh + 64],
                                pt[:, 0:64],
                                rcol[:],
                            )

                for qt in range(16):
                    nc.sync.dma_start(out=out_r[:, qt, :], in_=o_sb[:, qt, :])

    nc.compile()
    return nc


def _get_program():
    if "nc" not in _CACHE:
        _CACHE["nc"] = _build_program()
    return _CACHE["nc"]


def kernel(hidden_states, attention_mask, q_w, q_b, k_w, k_b, v_w, v_b):
    from concourse import bass_utils

    nc = _get_program()

    hs = np.asarray(hidden_states, np.float32)
    am = np.asarray(attention_mask, np.float32)
    q_w = np.asarray(q_w, np.float32)
    k_w = np.asarray(k_w, np.float32)
    v_w = np.asarray(v_w, np.float32)
    q_b = np.asarray(q_b, np.float32)
    k_b = np.asarray(k_b, np.float32)
    v_b = np.asarray(v_b, np.float32)

    scale = np.float32(1.0 / np.sqrt(HD))

    in_maps = []
    for c in range(NCORES):
        b = c // 4
        hg = c % 4
        cols = slice(WCOLS * hg, WCOLS * hg + WCOLS)
        mask = am[b, 0, 0, :]  # [S]
        in_maps.append(
            {
                "xt": np.ascontiguousarray(hs[b].T),
                "wq": np.ascontiguousarray(q_w[:, cols] * scale),
                "wk": np.ascontiguousarray(k_w[:, cols]),
                "wv": np.ascontiguousarray(v_w[:, cols]),
                "qb2": np.ascontiguousarray((q_b[cols] * scale).reshape(2, 128).T),
                "kb2": np.ascontiguousarray(k_b[cols].reshape(2, 128).T),
                "vb": np.ascontiguousarray(v_b[cols].reshape(1, WCOLS)),
                "maskc": np.ascontiguousarray(mask.reshape(16, 128).T),
            }
        )

    res = bass_utils.run_bass_kernel_spmd(nc, in_maps, core_ids=list(range(NCORES)))

    full = np.empty((B, S, HIDDEN), np.float32)
    for c in range(NCORES):
        b = c // 4
        hg = c % 4
        full[b, :, WCOLS * hg : WCOLS * hg + WCOLS] = res.results[c]["out"]
    return full


# revision 11
# speedup vs baseline: 1.5432x; 1.0127x over previous
"""BertSelfAttention on 8 Trainium2 NeuronCores.

Sharding: data parallel over batch (B=2) x tensor parallel over heads
(16 heads -> 4 groups of 4). Core c handles batch c//4, heads 4*(c%4)..+4.
No collectives needed: each core produces a disjoint [256, 2048] slice of
the output (feature-major) which the host transposes/concatenates.

Per-core device program (identical on all cores, SPMD over data):
  inputs (host-prepped):
    xt    [1024, 2048]  hidden_states[b].T          (f32r)
    wq/wk/wv [1024, 256] weight column slices (wq,qb pre-scaled by 1/8)
    qb2/kb2 [128, 2]    bias chunks (per-partition layout)
    vb    [1, 256]
    maskc [128, 16]     additive mask chunks (mask[c*128+p] at [p, c])
  output:
    out   [256, 2048]   context slice, feature-major (host transposes)

  Stage A (projections, PE, f32r single-pass matmuls):
    Q.T, K.T feature-major  [128 feats(2 heads), 2048 tokens]
    V token-major [128 tokens x 16 tiles, 4*(64+ones col)]  (bf16)
  Stage B (attention per head):
    S_T[k, q] = K_h-tile.T @ Q_h   (PE, f32r, contraction over d=64)
    expS = exp(S_T + mask_k)       (ACT, mask as per-partition bias, ->bf16)
    ctxT/denom = V_aug-tile.T @ expS summed over k  (PE bf16, M=65:
                 rows 0-63 unnormalized ctx.T, row 64 softmax denom)
    normalize: DVE reciprocal of denom row + DMA partition-broadcast +
    DVE multiply; result stays feature-major [64, 2048] per head.
"""

import numpy as np

HIDDEN = 1024
HEADS = 16
HD = 64
B = 2
S = 2048
NCORES = 8
HPC = HEADS // 4  # heads per core = 4
WCOLS = HPC * HD  # 256 weight columns per core

_CACHE = {}


def _build_program():
    import concourse.bass as bass
    import concourse.bacc as bacc
    import concourse.tile as tile
    import concourse.mybir as mybir

    f32 = mybir.dt.float32
    f32r = mybir.dt.float32r
    bf16 = mybir.dt.bfloat16

    nc = bacc.Bacc("TRN2", target_bir_lowering=False, debug=False, num_devices=NCORES)

    xt_d = nc.dram_tensor("xt", [HIDDEN, S], f32r, kind="ExternalInput")
    wq_d = nc.dram_tensor("wq", [HIDDEN, WCOLS], f32r, kind="ExternalInput")
    wk_d = nc.dram_tensor("wk", [HIDDEN, WCOLS], f32r, kind="ExternalInput")
    wv_d = nc.dram_tensor("wv", [HIDDEN, WCOLS], f32r, kind="ExternalInput")
    qb_d = nc.dram_tensor("qb2", [128, 2], f32, kind="ExternalInput")
    kb_d = nc.dram_tensor("kb2", [128, 2], f32, kind="ExternalInput")
    vb_d = nc.dram_tensor("vb", [1, WCOLS], f32, kind="ExternalInput")
    mask_d = nc.dram_tensor("maskc", [128, 16], f32, kind="ExternalInput")
    out_d = nc.dram_tensor("out", [WCOLS, S], f32, kind="ExternalOutput")

    xt_r = xt_d.ap().rearrange("(c p) m -> p c m", p=128)  # [128, 8, 2048]
    wq_r = wq_d.ap().rearrange("(c p) n -> p c n", p=128)  # [128, 8, 256]
    wk_r = wk_d.ap().rearrange("(c p) n -> p c n", p=128)
    wv_r = wv_d.ap().rearrange("(c p) n -> p c n", p=128)

    with tile.TileContext(nc) as tc:
        with tc.tile_pool(name="persist", bufs=1) as persist:
            # persistent SBUF
            q_sb = persist.tile([128, 2, S], f32r)  # [feat(2 heads), mc, token]
            k_sb = persist.tile([128, 2, S], f32r)
            v_sb = persist.tile([128, 16, 4 * 65], f32r)  # [token, tile, 4*(64+one)]
            qb_sb = persist.tile([128, 2], f32)
            kb_sb = persist.tile([128, 2], f32)
            vb_sb = persist.tile([1, WCOLS], f32)
            mask_sb = persist.tile([128, 16], f32)
            ones_sb = persist.tile([1, 128], f32)

            nc.sync.dma_start(out=qb_sb[:], in_=qb_d.ap())
            nc.sync.dma_start(out=kb_sb[:], in_=kb_d.ap())
            nc.sync.dma_start(out=vb_sb[:], in_=vb_d.ap())
            nc.sync.dma_start(out=mask_sb[:], in_=mask_d.ap())
            nc.vector.memset(ones_sb[:], 1.0)
            # ones columns of V (one per head): cols 64, 129, 194, 259
            # (memset can't write reduced dtypes here; round via DVE copy)
            v_blk = v_sb.rearrange("p m (l c) -> p m l c", l=4)
            vst = persist.tile([128, 16, 4], f32)
            nc.vector.memset(vst[:], 1.0)
            nc.vector.tensor_copy(v_blk[:, :, :, 64], vst[:])

            # ---------------- Stage A: projections ----------------
            with (
                tc.tile_pool(name="proj", bufs=1) as proj,
                tc.tile_pool(name="psA", bufs=3, space="PSUM") as psA,
                tc.tile_pool(name="psV", bufs=2, space="PSUM") as psV,
            ):
                xt = proj.tile([128, 8, S], f32r)
                wq_sb = proj.tile([128, 8, WCOLS], f32r)
                wk_sb = proj.tile([128, 8, WCOLS], f32r)
                wv_sb = proj.tile([128, 8, WCOLS], f32r)
                for k in range(8):
                    nc.sync.dma_start(out=xt[:, k, :], in_=xt_r[:, k, :])
                nc.sync.dma_start(out=wq_sb[:], in_=wq_r)
                nc.sync.dma_start(out=wk_sb[:], in_=wk_r)
                nc.sync.dma_start(out=wv_sb[:], in_=wv_r)

                # Q.T / K.T: [feat, token], feature-chunk mc, token-span sp
                for w_sb, b_sb, dst in ((wq_sb, qb_sb, q_sb), (wk_sb, kb_sb, k_sb)):
                    for mc in range(2):
                        for sp in range(4):
                            pq = psA.tile([128, 512], f32, tag="pq")
                            for k in range(8):
                                nc.tensor.matmul(
                                    pq[:],
                                    lhsT=w_sb[:, k, mc * 128 : mc * 128 + 128],
                                    rhs=xt[:, k, sp * 512 : sp * 512 + 512],
                                    start=(k == 0),
                                    stop=(k == 7),
                                )
                            nc.vector.tensor_scalar_add(
                                dst[:, mc, sp * 512 : sp * 512 + 512],
                                pq[:],
                                b_sb[:, mc : mc + 1],
                            )

                # V: token-major [token, feat]
                for mt in range(16):
                    pv = psV.tile([128, WCOLS], f32, tag="pv")
                    for k in range(8):
                        nc.tensor.matmul(
                            pv[:],
                            lhsT=xt[:, k, mt * 128 : mt * 128 + 128],
                            rhs=wv_sb[:, k, :],
                            start=(k == 0),
                            stop=False,
                        )
                    nc.tensor.matmul(
                        pv[:],
                        lhsT=ones_sb[0:1, 0:128],
                        rhs=vb_sb[0:1, :],
                        start=False,
                        stop=True,
                    )
                    for lh in range(4):
                        nc.vector.tensor_copy(
                            v_sb[:, mt, 65 * lh : 65 * lh + 64],
                            pv[:, 64 * lh : 64 * lh + 64],
                        )

            # ---------------- Stage B: attention ----------------
            with (
                tc.tile_pool(name="att", bufs=2) as att,
                tc.tile_pool(name="ctxp", bufs=2) as ctxp,
                tc.tile_pool(name="wkp", bufs=3) as wkp,
                tc.tile_pool(name="ps_s", bufs=3, space="PSUM") as ps_s,
                tc.tile_pool(name="ps_c", bufs=2, space="PSUM") as ps_c,
            ):
                for lh in range(4):
                    mc = lh // 2
                    rs = 64 * (lh % 2)
                    ctxn = ctxp.tile([64, S], f32, tag="ctxn")
                    for sp in range(4):
                        qs = sp * 512
                        expS = att.tile([128, 16, 512], f32r, tag="expS")
                        for kt in range(16):
                            ps = ps_s.tile([128, 512], f32, tag="ps")
                            nc.tensor.matmul(
                                ps[:],
                                lhsT=k_sb[rs : rs + 64, mc, kt * 128 : kt * 128 + 128],
                                rhs=q_sb[rs : rs + 64, mc, qs : qs + 512],
                                start=True,
                                stop=True,
                            )
                            nc.scalar.activation(
                                expS[:, kt, :],
                                ps[:],
                                mybir.ActivationFunctionType.Exp,
                                bias=mask_sb[:, kt : kt + 1],
                            )
                        # ctx.T + denominator, then normalize
                        pc = ps_c.tile([65, 512], f32, tag="pc")
                        for kt in range(16):
                            nc.tensor.matmul(
                                pc[:],
                                lhsT=v_sb[:, kt, 65 * lh : 65 * lh + 65],
                                rhs=expS[:, kt, :],
                                start=(kt == 0),
                                stop=(kt == 15),
                            )
                        den64 = wkp.tile([65, 512], f32, tag="den64")
                        nc.vector.tensor_copy(den64[64:65, :], pc[64:65, :])
                        den0 = wkp.tile([1, 512], f32, tag="den0")
                        nc.sync.dma_start(out=den0[:], in_=den64[64:65, :])
                        rden = wkp.tile([1, 512], f32, tag="rden")
                        nc.vector.reciprocal(rden[:], den0[:])
                        bc = wkp.tile([64, 512], f32, tag="bc")
                        nc.gpsimd.partition_broadcast(bc[:], rden[:])
                        nc.vector.tensor_mul(
                            ctxn[:, qs : qs + 512],
                            pc[0:64, :],
                            bc[:],
                        )
                    nc.sync.dma_start(
                        out=out_d.ap()[64 * lh : 64 * lh + 64, :], in_=ctxn[:]
                    )

    nc.compile()
    return nc


def _get_program():
    if "nc" not in _CACHE:
        _CACHE["nc"] = _build_program()
    return _CACHE["nc"]


def _make_in_maps(hidden_states, attention_mask, q_w, q_b, k_w, k_b, v_w, v_b):
    hs = np.asarray(hidden_states, np.float32)
    am = np.asarray(attention_mask, np.float32)
    q_w = np.asarray(q_w, np.float32)
    k_w = np.asarray(k_w, np.float32)
    v_w = np.asarray(v_w, np.float32)
    q_b = np.asarray(q_b, np.float32)
    k_b = np.asarray(k_b, np.float32)
    v_b = np.asarray(v_b, np.float32)

    scale = np.float32(1.0 / np.sqrt(HD))

    in_maps = []
    for c in range(NCORES):
        b = c // 4
        hg = c % 4
        cols = slice(WCOLS * hg, WCOLS * hg + WCOLS)
        mask = am[b, 0, 0, :]  # [S]
        in_maps.append(
            {
                "xt": np.ascontiguousarray(hs[b].T),
                "wq": np.ascontiguousarray(q_w[:, cols] * scale),
                "wk": np.ascontiguousarray(k_w[:, cols]),
                "wv": np.ascontiguousarray(v_w[:, cols]),
                "qb2": np.ascontiguousarray((q_b[cols] * scale).reshape(2, 128).T),
                "kb2": np.ascontiguousarray(k_b[cols].reshape(2, 128).T),
                "vb": np.ascontiguousarray(v_b[cols].reshape(1, WCOLS)),
                "maskc": np.ascontiguousarray(mask.reshape(16, 128).T),
            }
        )
    return in_maps


def kernel(hidden_states, attention_mask, q_w, q_b, k_w, k_b, v_w, v_b):
    from concourse import bass_utils

    nc = _get_program()
    in_maps = _make_in_maps(
        hidden_states, attention_mask, q_w, q_b, k_w, k_b, v_w, v_b
    )
    res = bass_utils.run_bass_kernel_spmd(nc, in_maps, core_ids=list(range(NCORES)))

    full = np.empty((B, S, HIDDEN), np.float32)
    for c in range(NCORES):
        b = c // 4
        hg = c % 4
        full[b, :, WCOLS * hg : WCOLS * hg + WCOLS] = res.results[c]["out"].T
    return full


# revision 13
# speedup vs baseline: 2.1532x; 1.3953x over previous
"""BertSelfAttention on 8 Trainium2 NeuronCores.

Sharding: data parallel over batch (B=2) x tensor parallel over heads
(16 heads -> 4 groups of 4). Core c handles batch c//4, heads 4*(c%4)..+4.
No collectives needed: each core produces a disjoint [256, 2048] slice of
the output (feature-major) which the host transposes/concatenates.

Per-core device program (identical on all cores, SPMD over data):
  inputs (host-prepped):
    xt    [1024, 2048]  hidden_states[b].T          (f32r)
    wq/wk/wv [1024, 256] weight column slices (wq,qb pre-scaled by 1/8)
    qb2/kb2 [128, 2]    bias chunks (per-partition layout)
    vb    [1, 256]
    maskc [128, 16]     additive mask chunks (mask[c*128+p] at [p, c])
  output:
    out   [256, 2048]   context slice, feature-major (host transposes)

  Stage A (projections, PE, f32r single-pass matmuls):
    Q.T, K.T feature-major  [128 feats(2 heads), 2048 tokens]
    V token-major [128 tokens x 16 tiles, 4*(64+ones col)]  (bf16)
  Stage B (attention per head):
    S_T[k, q] = K_h-tile.T @ Q_h   (PE, f32r, contraction over d=64)
    expS = exp(S_T + mask_k)       (ACT, mask as per-partition bias, ->bf16)
    ctxT/denom = V_aug-tile.T @ expS summed over k  (PE bf16, M=65:
                 rows 0-63 unnormalized ctx.T, row 64 softmax denom)
    normalize: DVE reciprocal of denom row + DMA partition-broadcast +
    DVE multiply; result stays feature-major [64, 2048] per head.
"""

import numpy as np

HIDDEN = 1024
HEADS = 16
HD = 64
B = 2
S = 2048
NCORES = 8
HPC = HEADS // 4  # heads per core = 4
WCOLS = HPC * HD  # 256 weight columns per core

_CACHE = {}


def _build_program():
    import concourse.bass as bass
    import concourse.bacc as bacc
    import concourse.tile as tile
    import concourse.mybir as mybir

    f32 = mybir.dt.float32
    f32r = mybir.dt.float32r
    bf16 = mybir.dt.bfloat16

    nc = bacc.Bacc("TRN2", target_bir_lowering=False, debug=False, num_devices=NCORES)

    xt_d = nc.dram_tensor("xt", [HIDDEN, S], f32r, kind="ExternalInput")
    wq_d = nc.dram_tensor("wq", [HIDDEN, WCOLS], f32r, kind="ExternalInput")
    wk_d = nc.dram_tensor("wk", [HIDDEN, WCOLS], f32r, kind="ExternalInput")
    wv_d = nc.dram_tensor("wv", [HIDDEN, WCOLS], f32r, kind="ExternalInput")
    qb_d = nc.dram_tensor("qb2", [128, 2], f32, kind="ExternalInput")
    kb_d = nc.dram_tensor("kb2", [128, 2], f32, kind="ExternalInput")
    vb_d = nc.dram_tensor("vb", [1, WCOLS], f32, kind="ExternalInput")
    mask_d = nc.dram_tensor("maskc", [128, 16], f32, kind="ExternalInput")
    out_d = nc.dram_tensor("out", [WCOLS, S], f32, kind="ExternalOutput")

    xt_r = xt_d.ap().rearrange("(c p) m -> p c m", p=128)  # [128, 8, 2048]
    wq_r = wq_d.ap().rearrange("(c p) n -> p c n", p=128)  # [128, 8, 256]
    wk_r = wk_d.ap().rearrange("(c p) n -> p c n", p=128)
    wv_r = wv_d.ap().rearrange("(c p) n -> p c n", p=128)

    with tile.TileContext(nc) as tc:
        with tc.tile_pool(name="persist", bufs=1) as persist:
            # persistent SBUF
            q_sb = persist.tile([128, 2, S], f32r)  # [feat(2 heads), mc, token]
            k_sb = persist.tile([128, 2, S], f32r)
            v_sb = persist.tile([128, 16, 4 * 65], f32r)  # [token, tile, 4*(64+one)]
            qb_sb = persist.tile([128, 2], f32)
            kb_sb = persist.tile([128, 2], f32)
            vb_sb = persist.tile([1, WCOLS], f32)
            mask_sb = persist.tile([128, 16], f32)
            ones_sb = persist.tile([1, 128], f32)

            nc.sync.dma_start(out=qb_sb[:], in_=qb_d.ap())
            nc.sync.dma_start(out=kb_sb[:], in_=kb_d.ap())
            nc.sync.dma_start(out=vb_sb[:], in_=vb_d.ap())
            nc.sync.dma_start(out=mask_sb[:], in_=mask_d.ap())
            nc.vector.memset(ones_sb[:], 1.0)
            # ones columns of V (one per head): cols 64, 129, 194, 259
            # (memset can't write reduced dtypes here; round via DVE copy)
            v_blk = v_sb.rearrange("p m (l c) -> p m l c", l=4)
            vst = persist.tile([128, 16, 4], f32)
            nc.vector.memset(vst[:], 1.0)
            nc.vector.tensor_copy(v_blk[:, :, :, 64], vst[:])

            # ---------------- Stage A: projections ----------------
            with (
                tc.tile_pool(name="proj", bufs=1) as proj,
                tc.tile_pool(name="psA", bufs=3, space="PSUM") as psA,
                tc.tile_pool(name="psV", bufs=2, space="PSUM") as psV,
            ):
                xt = proj.tile([128, 8, S], f32r)
                wq_sb = proj.tile([128, 8, WCOLS], f32r)
                wk_sb = proj.tile([128, 8, WCOLS], f32r)
                wv_sb = proj.tile([128, 8, WCOLS], f32r)
                for k in range(8):
                    nc.sync.dma_start(out=xt[:, k, :], in_=xt_r[:, k, :])
                nc.sync.dma_start(out=wq_sb[:], in_=wq_r)
                nc.sync.dma_start(out=wk_sb[:], in_=wk_r)
                nc.sync.dma_start(out=wv_sb[:], in_=wv_r)

                # Q.T / K.T: [feat, token], feature-chunk mc, token-span sp
                for w_sb, b_sb, dst in ((wq_sb, qb_sb, q_sb), (wk_sb, kb_sb, k_sb)):
                    for mc in range(2):
                        for sp in range(4):
                            pq = psA.tile([128, 512], f32, tag="pq")
                            for k in range(8):
                                nc.tensor.matmul(
                                    pq[:],
                                    lhsT=w_sb[:, k, mc * 128 : mc * 128 + 128],
                                    rhs=xt[:, k, sp * 512 : sp * 512 + 512],
                                    start=(k == 0),
                                    stop=(k == 7),
                                )
                            nc.vector.tensor_scalar_add(
                                dst[:, mc, sp * 512 : sp * 512 + 512],
                                pq[:],
                                b_sb[:, mc : mc + 1],
                            )

                # V: token-major [token, feat]
                for mt in range(16):
                    pv = psV.tile([128, WCOLS], f32, tag="pv")
                    for k in range(8):
                        nc.tensor.matmul(
                            pv[:],
                            lhsT=xt[:, k, mt * 128 : mt * 128 + 128],
                            rhs=wv_sb[:, k, :],
                            start=(k == 0),
                            stop=False,
                        )
                    nc.tensor.matmul(
                        pv[:],
                        lhsT=ones_sb[0:1, 0:128],
                        rhs=vb_sb[0:1, :],
                        start=False,
                        stop=True,
                    )
                    for lh in range(4):
                        nc.vector.tensor_copy(
                            v_sb[:, mt, 65 * lh : 65 * lh + 64],
                            pv[:, 64 * lh : 64 * lh + 64],
                        )

            # ---------------- Stage B: attention ----------------
            # Heads processed in pairs (mc): S_T for both heads of the pair
            # runs as two row-packed concurrent matmuls (rows 0-63 / 64-127),
            # exp evacuates the merged [128,1024] psum in one ACT op.
            with (
                tc.tile_pool(name="att", bufs=2) as att,
                tc.tile_pool(name="ctxp", bufs=3) as ctxp,
                tc.tile_pool(name="wkp", bufs=2) as wkp,
                tc.tile_pool(name="ps_s", bufs=2, space="PSUM") as ps_s,
                tc.tile_pool(name="ps_c", bufs=2, space="PSUM") as ps_c,
            ):
                for mc in range(2):
                    for sp in range(4):
                        qs = sp * 512
                        # expP[:, kt, 0:512] = head 2mc, [:, kt, 512:1024] = head 2mc+1
                        expP = att.tile([128, 16, 1024], f32r, tag="expP")
                        for kt in range(16):
                            ps = ps_s.tile([128, 1024], f32, tag="ps")
                            for half in range(2):
                                rs = 64 * half
                                nc.tensor.matmul(
                                    ps[:, half * 512 : half * 512 + 512],
                                    lhsT=k_sb[
                                        rs : rs + 64, mc, kt * 128 : kt * 128 + 128
                                    ],
                                    rhs=q_sb[rs : rs + 64, mc, qs : qs + 512],
                                    start=True,
                                    stop=True,
                                )
                            nc.scalar.activation(
                                expP[:, kt, :],
                                ps[:],
                                mybir.ActivationFunctionType.Exp,
                                bias=mask_sb[:, kt : kt + 1],
                            )
                        for half in range(2):
                            lh = 2 * mc + half
                            pc = ps_c.tile([65, 512], f32, tag="pc")
                            for kt in range(16):
                                nc.tensor.matmul(
                                    pc[:],
                                    lhsT=v_sb[:, kt, 65 * lh : 65 * lh + 65],
                                    rhs=expP[:, kt, half * 512 : half * 512 + 512],
                                    start=(kt == 0),
                                    stop=(kt == 15),
                                )
                            den64 = wkp.tile([65, 512], f32, tag="den64")
                            nc.vector.tensor_copy(den64[64:65, :], pc[64:65, :])
                            den0 = wkp.tile([1, 512], f32, tag="den0")
                            nc.sync.dma_start(out=den0[:], in_=den64[64:65, :])
                            rden = wkp.tile([1, 512], f32, tag="rden")
                            nc.vector.reciprocal(rden[:], den0[:])
                            bc = wkp.tile([64, 512], f32, tag="bc")
                            nc.gpsimd.partition_broadcast(bc[:], rden[:])
                            ctxs = ctxp.tile([64, 512], f32, tag="ctxs")
                            nc.vector.tensor_mul(ctxs[:], pc[0:64, :], bc[:])
                            nc.sync.dma_start(
                                out=out_d.ap()[64 * lh : 64 * lh + 64, qs : qs + 512],
                                in_=ctxs[:],
                            )

    nc.compile()
    return nc


def _get_program():
    if "nc" not in _CACHE:
        _CACHE["nc"] = _build_program()
    return _CACHE["nc"]


def _make_in_maps(hidden_states, attention_mask, q_w, q_b, k_w, k_b, v_w, v_b):
    hs = np.asarray(hidden_states, np.float32)
    am = np.asarray(attention_mask, np.float32)
    q_w = np.asarray(q_w, np.float32)
    k_w = np.asarray(k_w, np.float32)
    v_w = np.asarray(v_w, np.float32)
    q_b = np.asarray(q_b, np.float32)
    k_b = np.asarray(k_b, np.float32)
    v_b = np.asarray(v_b, np.float32)

    scale = np.float32(1.0 / np.sqrt(HD))

    in_maps = []
    for c in range(NCORES):
        b = c // 4
        hg = c % 4
        cols = slice(WCOLS * hg, WCOLS * hg + WCOLS)
        mask = am[b, 0, 0, :]  # [S]
        in_maps.append(
            {
                "xt": np.ascontiguousarray(hs[b].T),
                "wq": np.ascontiguousarray(q_w[:, cols] * scale),
                "wk": np.ascontiguousarray(k_w[:, cols]),
                "wv": np.ascontiguousarray(v_w[:, cols]),
                "qb2": np.ascontiguousarray((q_b[cols] * scale).reshape(2, 128).T),
                "kb2": np.ascontiguousarray(k_b[cols].reshape(2, 128).T),
                "vb": np.ascontiguousarray(v_b[cols].reshape(1, WCOLS)),
                "maskc": np.ascontiguousarray(mask.reshape(16, 128).T),
            }
        )
    return in_maps


def kernel(hidden_states, attention_mask, q_w, q_b, k_w, k_b, v_w, v_b):
    from concourse import bass_utils

    nc = _get_program()
    in_maps = _make_in_maps(
        hidden_states, attention_mask, q_w, q_b, k_w, k_b, v_w, v_b
    )
    res = bass_utils.run_bass_kernel_spmd(nc, in_maps, core_ids=list(range(NCORES)))

    full = np.empty((B, S, HIDDEN), np.float32)
    for c in range(NCORES):
        b = c // 4
        hg = c % 4
        full[b, :, WCOLS * hg : WCOLS * hg + WCOLS] = res.results[c]["out"].T
    return full
